# revision 3
# baseline (speedup 1.0000x reference)
"""Trainium2 Bass kernel for per-image 3x3 Gaussian blur (AddingGaussianBlur).

The reference op (with its faithful ys=xs bug) reduces to a separable filter:
  out[b,h,w,c] = sum_j h_j(b) * V[b, h, w+j-1, c],   j in {0,1,2}
  V[b,h,w,c]   = x[b,h-1,w,c] + x[b,h,w,c] + x[b,h+1,w,c]   (zero padded)
  h_0 = h_2 = e / (3*(1+2e)),  h_1 = 1 / (3*(1+2e)),  e = exp(-1/(3*std)^2)

Device strategy (pure data parallel, 8 images per core):
  - Layout per image: SBUF tile (128 partitions = rows mod 128, 4 row-blocks x
    1542 cols) in bf16, zero-padded 3 elements (1 pixel) on each side of the
    1536-wide (w,c) axis.  Loaded with a casting SWDGE DMA (f32 -> bf16).
  - Vertical [1,1,1] box + horizontal taps fused into 3 accumulating
    TensorEngine matmuls per PSUM bank: stationary = h_j * tridiag(128),
    moving = the padded tile at element shifts {0, 3, 6}.
  - Cross-block vertical halo rows are horizontally prefiltered on the host
    (1.2% of the data) and added with one K=2 matmul per bank against a
    one-hot (2,128) selector.
  - ScalarE copies PSUM (f32) -> SBUF, HWDGE DMA stores f32 to HBM.
"""

import os

import numpy as np

import concourse.bass as bass  # noqa: F401  (bass types referenced indirectly)
import concourse.tile as tile
from concourse import bacc, mybir
from concourse.bass_utils import run_bass_kernel_spmd

N_CORES = 8
B = 64
BPC = B // N_CORES  # images per core
H, W, C = 512, 512, 3
F = W * C  # 1536
P = 128
NB = H // P  # 4 row-blocks per image
PAD = 3  # one pixel of (w,c) padding
FP = F + 2 * PAD  # 1542
BANK = 512  # fp32 elements per PSUM bank
NBANK = F // BANK  # 3

LAST_RESULTS = None  # BassKernelResults of the most recent run (for test.py)


def _build_nc():
    f32 = mybir.dt.float32
    bf16 = mybir.dt.bfloat16
    nc = bacc.Bacc("TRN2", target_bir_lowering=False, debug=False)

    x = nc.declare_dram_parameter("x", [BPC, H, F], f32, isOutput=False)
    hbnd = nc.declare_dram_parameter("hbnd", [2, BPC, NB, F], f32, isOutput=False)
    hwts = nc.declare_dram_parameter("hwts", [P, 2 * BPC], f32, isOutput=False)
    tri = nc.declare_dram_parameter("tri", [P, P], f32, isOutput=False)
    bsel = nc.declare_dram_parameter("bsel", [2, P], f32, isOutput=False)
    out = nc.declare_dram_parameter("out", [BPC, H, F], f32, isOutput=True)

    # row h = 128*n + p  ->  partition p, free block n
    xr = x[:].rearrange("b (n p) f -> b p n f", p=P)
    outr = out[:].rearrange("b (n p) f -> b p n f", p=P)
    hbr = hbnd[:]

    with tile.TileContext(nc) as tc:
        with (
            tc.tile_pool(name="const", bufs=1) as cpool,
            tc.tile_pool(name="xin", bufs=8) as xpool,
            tc.tile_pool(name="pbin", bufs=2) as pbpool,
            tc.tile_pool(name="oout", bufs=4) as opool,
            tc.tile_pool(name="ps", bufs=2, space="PSUM") as ppool,
        ):
            # Constants via HWDGE (keeps the SWDGE queue free for the big
            # input loads); bf16 casts done on the idle VectorE.
            tri_f = cpool.tile([P, P], f32, name="tri_f")
            nc.sync.dma_start(out=tri_f, in_=tri[:])
            bsel_f = cpool.tile([2, P], f32, name="bsel_f")
            nc.sync.dma_start(out=bsel_f, in_=bsel[:])
            hw_sb = cpool.tile([P, 2 * BPC], f32, name="hw_sb")
            nc.sync.dma_start(out=hw_sb, in_=hwts[:])
            tri_bf = cpool.tile([P, P], bf16, name="tri_bf")
            nc.vector.tensor_copy(out=tri_bf, in_=tri_f)
            bsel_bf = cpool.tile([2, P], bf16, name="bsel_bf")
            nc.vector.tensor_copy(out=bsel_bf, in_=bsel_f)

            # Per-image stationaries: h0 * tridiag and h1 * tridiag (bf16)
            tws = []
            for i in range(BPC):
                t0 = cpool.tile([P, P], bf16, name=f"tw0_{i}", tag=f"tw0_{i}")
                nc.vector.tensor_scalar_mul(
                    out=t0, in0=tri_bf, scalar1=hw_sb[:, 2 * i : 2 * i + 1]
                )
                t1 = cpool.tile([P, P], bf16, name=f"tw1_{i}", tag=f"tw1_{i}")
                nc.vector.tensor_scalar_mul(
                    out=t1, in0=tri_bf, scalar1=hw_sb[:, 2 * i + 1 : 2 * i + 2]
                )
                tws.append((t0, t1))

            for i in range(BPC):
                pb = pbpool.tile([2, NB, F], bf16, name="pb")
                nc.gpsimd.dma_start(out=pb, in_=hbr[:, i])

                xis = []
                for n in range(NB):
                    xi = xpool.tile([P, FP], bf16, name="xi")
                    nc.vector.memset(xi[:, 0:PAD], 0.0)
                    nc.vector.memset(xi[:, F + PAD : FP], 0.0)
                    nc.gpsimd.dma_start(out=xi[:, PAD : F + PAD], in_=xr[i][:, n, :])
                    xis.append(xi)

                t0, t1 = tws[i]
                for n in range(NB):
                    xi = xis[n]
                    pt = ppool.tile([P, F], f32, name="pt")
                    # taps at shifts 0 and 6 share the h0 stationary
                    for b in range(NBANK):
                        nc.tensor.matmul(
                            out=pt[:, b * BANK : (b + 1) * BANK],
                            lhsT=t0,
                            rhs=xi[:, b * BANK : b * BANK + BANK],
                            start=True,
                            stop=False,
                        )
                    for b in range(NBANK):
                        nc.tensor.matmul(
                            out=pt[:, b * BANK : (b + 1) * BANK],
                            lhsT=t0,
                            rhs=xi[:, b * BANK + 6 : b * BANK + 6 + BANK],
                            start=False,
                            stop=False,
                        )
                    for b in range(NBANK):
                        nc.tensor.matmul(
                            out=pt[:, b * BANK : (b + 1) * BANK],
                            lhsT=t1,
                            rhs=xi[:, b * BANK + 3 : b * BANK + 3 + BANK],
                            start=False,
                            stop=False,
                        )
                    for b in range(NBANK):
                        nc.tensor.matmul(
                            out=pt[:, b * BANK : (b + 1) * BANK],
                            lhsT=bsel_bf,
                            rhs=pb[:, n, b * BANK : (b + 1) * BANK],
                            start=False,
                            stop=True,
                        )
                    oi = opool.tile([P, F], f32, name="oi")
                    nc.scalar.copy(out=oi, in_=pt)
                    nc.sync.dma_start(out=outr[i][:, n, :], in_=oi)

    nc.compile()
    return nc


def _hfilt(rows, h0, h1):
    """Horizontal 3-tap filter of full-width rows. rows: (B', W, C) f32."""
    p = np.pad(rows, ((0, 0), (1, 1), (0, 0)))
    return (
        h0[:, None, None] * (p[:, :-2] + p[:, 2:]) + h1[:, None, None] * rows
    ).astype(np.float32)


def kernel(x, stds):
    global LAST_RESULTS
    x = np.ascontiguousarray(np.asarray(x), dtype=np.float32)
    stds = np.asarray(stds, dtype=np.float32)
    assert x.shape == (B, H, W, C) and stds.shape == (B,)

    # Per-image horizontal tap weights (f32, mirrors the reference math)
    s = (stds * np.float32(3.0)).astype(np.float32)
    with np.errstate(divide="ignore", over="ignore"):
        e = np.exp(-(np.float32(1.0) / (s * s))).astype(np.float32)
    den = (np.float32(3.0) * (np.float32(1.0) + np.float32(2.0) * e)).astype(np.float32)
    h0 = (e / den).astype(np.float32)
    h1 = (np.float32(1.0) / den).astype(np.float32)

    xf = x.reshape(B, H, F)

    # Host-prefiltered vertical halo rows: hbnd[0,:,n] = H(x[:, 128n-1]),
    # hbnd[1,:,n] = H(x[:, 128(n+1)]); zero where the halo row is outside.
    hbnd = np.zeros((2, B, NB, F), np.float32)
    for n in range(1, NB):
        hbnd[0, :, n] = _hfilt(x[:, P * n - 1], h0, h1).reshape(B, F)
    for n in range(0, NB - 1):
        hbnd[1, :, n] = _hfilt(x[:, P * (n + 1)], h0, h1).reshape(B, F)

    tri_np = np.zeros((P, P), np.float32)
    idx = np.arange(P)
    tri_np[idx, idx] = 1.0
    tri_np[idx[:-1], idx[:-1] + 1] = 1.0
    tri_np[idx[:-1] + 1, idx[:-1]] = 1.0

    bsel_np = np.zeros((2, P), np.float32)
    bsel_np[0, 0] = 1.0
    bsel_np[1, P - 1] = 1.0

    in_maps = []
    for c in range(N_CORES):
        sl = slice(c * BPC, (c + 1) * BPC)
        hw_np = np.zeros((P, 2 * BPC), np.float32)
        hw_np[:, 0::2] = h0[sl][None, :]
        hw_np[:, 1::2] = h1[sl][None, :]
        in_maps.append(
            {
                "x": xf[sl],
                "hbnd": np.ascontiguousarray(hbnd[:, sl]),
                "hwts": hw_np,
                "tri": tri_np,
                "bsel": bsel_np,
            }
        )

    nc = _build_nc()
    trace = bool(int(os.environ.get("BLUR_TRACE", "0")))
    res = run_bass_kernel_spmd(
        nc, in_maps, core_ids=list(range(N_CORES)), trace=trace
    )
    LAST_RESULTS = res

    outs = [res.results[c]["out"].reshape(BPC, H, W, C) for c in range(N_CORES)]
    return np.concatenate(outs, axis=0).astype(np.float32)


# revision 7
# speedup vs baseline: 1.0373x; 1.0373x over previous
"""Trainium2 Bass kernel for per-image 3x3 Gaussian blur (AddingGaussianBlur).

The reference op (with its faithful ys=xs bug) reduces to a separable filter:
  out[b,h,w,c] = sum_j h_j(b) * V[b, h, w+j-1, c],   j in {0,1,2}
  V[b,h,w,c]   = x[b,h-1,w,c] + x[b,h,w,c] + x[b,h+1,w,c]   (zero padded)
  h_0 = h_2 = e / (3*(1+2e)),  h_1 = 1 / (3*(1+2e)),  e = exp(-1/(3*std)^2)

Device strategy (pure data parallel, 8 images per core):
  - Layout per image: SBUF tile (128 partitions = rows mod 128, 4 row-blocks x
    1542 cols) in bf16, zero-padded 3 elements (1 pixel) on each side of the
    1536-wide (w,c) axis.  Loaded with a casting SWDGE DMA (f32 -> bf16).
  - Vertical [1,1,1] box + horizontal taps fused into 3 accumulating
    TensorEngine matmuls per PSUM bank: stationary = h_j * tridiag(128),
    moving = the padded tile at element shifts {0, 3, 6}.
  - Cross-block vertical halo rows are horizontally prefiltered on the host
    (1.2% of the data) and added with one K=2 matmul per bank against a
    one-hot (2,128) selector.
  - ScalarE copies PSUM (f32) -> SBUF, HWDGE DMA stores f32 to HBM.
"""

import os

import numpy as np

import concourse.bass as bass  # noqa: F401  (bass types referenced indirectly)
import concourse.tile as tile
from concourse import bacc, mybir
from concourse.bass_utils import run_bass_kernel_spmd

N_CORES = 8
B = 64
BPC = B // N_CORES  # images per core
H, W, C = 512, 512, 3
F = W * C  # 1536
P = 128
NB = H // P  # 4 row-blocks per image
PAD = 3  # one pixel of (w,c) padding
FP = F + 2 * PAD  # 1542
BANK = 512  # fp32 elements per PSUM bank
NBANK = F // BANK  # 3

LAST_RESULTS = None  # BassKernelResults of the most recent run (for test.py)


def _build_nc():
    f32 = mybir.dt.float32
    bf16 = mybir.dt.bfloat16
    nc = bacc.Bacc("TRN2", target_bir_lowering=False, debug=False)

    x = nc.declare_dram_parameter("x", [BPC, H, F], f32, isOutput=False)
    hbnd = nc.declare_dram_parameter("hbnd", [2, BPC, NB, F], f32, isOutput=False)
    hwts = nc.declare_dram_parameter("hwts", [P, 2 * BPC], f32, isOutput=False)
    tri = nc.declare_dram_parameter("tri", [P, P], f32, isOutput=False)
    bsel = nc.declare_dram_parameter("bsel", [2, P], f32, isOutput=False)
    out = nc.declare_dram_parameter("out", [BPC, H, F], f32, isOutput=True)

    # row h = 128*n + p  ->  partition p, free block n
    xr = x[:].rearrange("b (n p) f -> b p n f", p=P)
    outr = out[:].rearrange("b (n p) f -> b p n f", p=P)
    hbr = hbnd[:]

    with tile.TileContext(nc) as tc:
        with (
            tc.tile_pool(name="const", bufs=1) as cpool,
            tc.tile_pool(name="xin", bufs=3) as xpool,
            tc.tile_pool(name="pbin", bufs=2) as pbpool,
            tc.tile_pool(name="oout", bufs=2) as opool,
            tc.tile_pool(name="ps", bufs=2, space="PSUM") as ppool,
        ):
            # Constants via HWDGE (keeps the SWDGE queue free for the big
            # input loads); bf16 casts done on the idle VectorE.
            tri_f = cpool.tile([P, P], f32, name="tri_f")
            nc.sync.dma_start(out=tri_f, in_=tri[:])
            bsel_f = cpool.tile([2, P], f32, name="bsel_f")
            nc.sync.dma_start(out=bsel_f, in_=bsel[:])
            hw_sb = cpool.tile([P, 2 * BPC], f32, name="hw_sb")
            nc.sync.dma_start(out=hw_sb, in_=hwts[:])
            tri_bf = cpool.tile([P, P], bf16, name="tri_bf")
            nc.vector.tensor_copy(out=tri_bf, in_=tri_f)
            bsel_bf = cpool.tile([2, P], bf16, name="bsel_bf")
            nc.vector.tensor_copy(out=bsel_bf, in_=bsel_f)

            # Per-image stationaries: h0 * tridiag and h1 * tridiag (bf16)
            tws = []
            for i in range(BPC):
                t0 = cpool.tile([P, P], bf16, name=f"tw0_{i}", tag=f"tw0_{i}")
                nc.vector.tensor_scalar_mul(
                    out=t0, in0=tri_bf, scalar1=hw_sb[:, 2 * i : 2 * i + 1]
                )
                t1 = cpool.tile([P, P], bf16, name=f"tw1_{i}", tag=f"tw1_{i}")
                nc.vector.tensor_scalar_mul(
                    out=t1, in0=tri_bf, scalar1=hw_sb[:, 2 * i + 1 : 2 * i + 2]
                )
                tws.append((t0, t1))

            for i in range(BPC):
                # Image 0: per-block loads so the PE can start ~3 MB/358GB/s
                # sooner; steady state: one 3 MB load per image (max DMA
                # efficiency).  Symmetrically, last image stores per block.
                if i == 0:
                    xi = xpool.tile([P, NB, FP], bf16, name="xi")
                    nc.vector.memset(xi[:, :, 0:PAD], 0.0)
                    nc.vector.memset(xi[:, :, F + PAD : FP], 0.0)
                    for n in range(NB):
                        nc.gpsimd.dma_start(
                            out=xi[:, n, PAD : F + PAD], in_=xr[i][:, n, :]
                        )
                else:
                    xi = xpool.tile([P, NB, FP], bf16, name="xi")
                    nc.vector.memset(xi[:, :, 0:PAD], 0.0)
                    nc.vector.memset(xi[:, :, F + PAD : FP], 0.0)
                    nc.gpsimd.dma_start(out=xi[:, :, PAD : F + PAD], in_=xr[i])

                pb = pbpool.tile([2, NB, F], bf16, name="pb")
                nc.gpsimd.dma_start(out=pb, in_=hbr[:, i])

                oi = None
                if i < BPC - 1:
                    oi = opool.tile([P, NB, F], f32, name="oi", tag="oi")

                t0, t1 = tws[i]
                for n in range(NB):
                    pt = ppool.tile([P, F], f32, name="pt")
                    # taps at shifts 0 and 6 share the h0 stationary
                    for b in range(NBANK):
                        nc.tensor.matmul(
                            out=pt[:, b * BANK : (b + 1) * BANK],
                            lhsT=t0,
                            rhs=xi[:, n, b * BANK : b * BANK + BANK],
                            start=True,
                            stop=False,
                        )
                    for b in range(NBANK):
                        nc.tensor.matmul(
                            out=pt[:, b * BANK : (b + 1) * BANK],
                            lhsT=t0,
                            rhs=xi[:, n, b * BANK + 6 : b * BANK + 6 + BANK],
                            start=False,
                            stop=False,
                        )
                    for b in range(NBANK):
                        nc.tensor.matmul(
                            out=pt[:, b * BANK : (b + 1) * BANK],
                            lhsT=t1,
                            rhs=xi[:, n, b * BANK + 3 : b * BANK + 3 + BANK],
                            start=False,
                            stop=False,
                        )
                    for b in range(NBANK):
                        nc.tensor.matmul(
                            out=pt[:, b * BANK : (b + 1) * BANK],
                            lhsT=bsel_bf,
                            rhs=pb[:, n, b * BANK : (b + 1) * BANK],
                            start=False,
                            stop=True,
                        )
                    if i < BPC - 1:
                        nc.scalar.copy(out=oi[:, n, :], in_=pt)
                    else:
                        # last image: per-block stores to shorten the tail
                        ob = opool.tile([P, F], f32, name="ob", tag="ob", bufs=4)
                        nc.scalar.copy(out=ob, in_=pt)
                        nc.sync.dma_start(out=outr[i][:, n, :], in_=ob)
                if i < BPC - 1:
                    nc.sync.dma_start(out=outr[i], in_=oi)

    nc.compile()
    return nc


def _hfilt(rows, h0, h1):
    """Horizontal 3-tap filter of full-width rows. rows: (B', W, C) f32."""
    p = np.pad(rows, ((0, 0), (1, 1), (0, 0)))
    return (
        h0[:, None, None] * (p[:, :-2] + p[:, 2:]) + h1[:, None, None] * rows
    ).astype(np.float32)


def kernel(x, stds):
    global LAST_RESULTS
    x = np.ascontiguousarray(np.asarray(x), dtype=np.float32)
    stds = np.asarray(stds, dtype=np.float32)
    assert x.shape == (B, H, W, C) and stds.shape == (B,)

    # Per-image horizontal tap weights (f32, mirrors the reference math)
    s = (stds * np.float32(3.0)).astype(np.float32)
    with np.errstate(divide="ignore", over="ignore"):
        e = np.exp(-(np.float32(1.0) / (s * s))).astype(np.float32)
    den = (np.float32(3.0) * (np.float32(1.0) + np.float32(2.0) * e)).astype(np.float32)
    h0 = (e / den).astype(np.float32)
    h1 = (np.float32(1.0) / den).astype(np.float32)

    xf = x.reshape(B, H, F)

    # Host-prefiltered vertical halo rows: hbnd[0,:,n] = H(x[:, 128n-1]),
    # hbnd[1,:,n] = H(x[:, 128(n+1)]); zero where the halo row is outside.
    hbnd = np.zeros((2, B, NB, F), np.float32)
    for n in range(1, NB):
        hbnd[0, :, n] = _hfilt(x[:, P * n - 1], h0, h1).reshape(B, F)
    for n in range(0, NB - 1):
        hbnd[1, :, n] = _hfilt(x[:, P * (n + 1)], h0, h1).reshape(B, F)

    tri_np = np.zeros((P, P), np.float32)
    idx = np.arange(P)
    tri_np[idx, idx] = 1.0
    tri_np[idx[:-1], idx[:-1] + 1] = 1.0
    tri_np[idx[:-1] + 1, idx[:-1]] = 1.0

    bsel_np = np.zeros((2, P), np.float32)
    bsel_np[0, 0] = 1.0
    bsel_np[1, P - 1] = 1.0

    in_maps = []
    for c in range(N_CORES):
        sl = slice(c * BPC, (c + 1) * BPC)
        hw_np = np.zeros((P, 2 * BPC), np.float32)
        hw_np[:, 0::2] = h0[sl][None, :]
        hw_np[:, 1::2] = h1[sl][None, :]
        in_maps.append(
            {
                "x": xf[sl],
                "hbnd": np.ascontiguousarray(hbnd[:, sl]),
                "hwts": hw_np,
                "tri": tri_np,
                "bsel": bsel_np,
            }
        )

    nc = _build_nc()
    trace = bool(int(os.environ.get("BLUR_TRACE", "0")))
    res = run_bass_kernel_spmd(
        nc, in_maps, core_ids=list(range(N_CORES)), trace=trace
    )
    LAST_RESULTS = res

    outs = [res.results[c]["out"].reshape(BPC, H, W, C) for c in range(N_CORES)]
    return np.concatenate(outs, axis=0).astype(np.float32)


# revision 13
# speedup vs baseline: 1.1590x; 1.1173x over previous
"""Trainium2 Bass kernel for per-image 3x3 Gaussian blur (AddingGaussianBlur).

The reference op (with its faithful ys=xs bug) reduces to a separable filter:
  out[b,h,w,c] = sum_j h_j(b) * V[b, h, w+j-1, c],   j in {0,1,2}
  V[b,h,w,c]   = x[b,h-1,w,c] + x[b,h,w,c] + x[b,h+1,w,c]   (zero padded)
  h_0 = h_2 = e / (3*(1+2e)),  h_1 = 1 / (3*(1+2e)),  e = exp(-1/(3*std)^2)

Device strategy (pure data parallel, 8 images per core):
  - Layout per image: SBUF tile (128 partitions = rows mod 128, 4 row-blocks x
    1542 cols) in bf16, zero-padded 3 elements (1 pixel) on each side of the
    1536-wide (w,c) axis.  Loaded with a casting SWDGE DMA (f32 -> bf16).
  - Vertical [1,1,1] box + horizontal taps fused into 3 accumulating
    TensorEngine matmuls per PSUM bank: stationary = h_j * tridiag(128),
    moving = the padded tile at element shifts {0, 3, 6}.
  - Cross-block vertical halo rows are horizontally prefiltered on the host
    (1.2% of the data) and added with one K=2 matmul per bank against a
    one-hot (2,128) selector.
  - ScalarE copies PSUM (f32) -> SBUF, HWDGE DMA stores f32 to HBM.
"""

import os

import numpy as np

import concourse.bass as bass  # noqa: F401  (bass types referenced indirectly)
import concourse.tile as tile
from concourse import bacc, mybir
from concourse.bass_utils import run_bass_kernel_spmd

N_CORES = 8
B = 64
BPC = B // N_CORES  # images per core
H, W, C = 512, 512, 3
F = W * C  # 1536
P = 128
NB = H // P  # 4 row-blocks per image
PAD = 3  # one pixel of (w,c) padding
FP = F + 2 * PAD  # 1542
BANK = 512  # fp32 elements per PSUM bank
NBANK = F // BANK  # 3

LAST_RESULTS = None  # BassKernelResults of the most recent run (for test.py)


def _build_nc():
    f32 = mybir.dt.float32
    bf16 = mybir.dt.bfloat16
    nc = bacc.Bacc("TRN2", target_bir_lowering=False, debug=False)

    x = nc.declare_dram_parameter("x", [BPC, H, F], f32, isOutput=False)
    # hbnd[i, n, 0] = prefiltered prev-halo row of block n, [i, n, 1] = next
    hbnd = nc.declare_dram_parameter("hbnd", [BPC, NB, 2, F], f32, isOutput=False)
    hwts = nc.declare_dram_parameter("hwts", [P, 2 * BPC], f32, isOutput=False)
    tri = nc.declare_dram_parameter("tri", [P, P], f32, isOutput=False)
    bsel = nc.declare_dram_parameter("bsel", [NB, 2, P], f32, isOutput=False)
    out = nc.declare_dram_parameter("out", [BPC, H, F], f32, isOutput=True)

    # row h = 128*n + p  ->  partition p, free block n
    xr = x[:].rearrange("b (n p) f -> b p n f", p=P)
    outr = out[:].rearrange("b (n p) f -> b p n f", p=P)
    hbr = hbnd[:]

    with tile.TileContext(nc) as tc:
        with (
            tc.tile_pool(name="const", bufs=1) as cpool,
            tc.tile_pool(name="xin", bufs=6) as xpool,
            tc.tile_pool(name="pbin", bufs=8) as pbpool,
            tc.tile_pool(name="oout", bufs=2) as opool,
            tc.tile_pool(name="ps", bufs=2, space="PSUM") as ppool,
        ):
            # Constants via HWDGE (keeps the SWDGE queue free for the big
            # input loads); bf16 casts done on the idle VectorE.
            tri_f = cpool.tile([P, P], f32, name="tri_f")
            nc.sync.dma_start(out=tri_f, in_=tri[:])
            hw_sb = cpool.tile([P, 2 * BPC], f32, name="hw_sb")
            nc.sync.dma_start(out=hw_sb, in_=hwts[:])
            tri_bf = cpool.tile([P, P], bf16, name="tri_bf")
            nc.vector.tensor_copy(out=tri_bf, in_=tri_f)
            # One-hot boundary selectors for each row-block, placed at
            # partitions 32n..32n+1 so K=2 matmuls get legal tile_positions.
            bsel_bf = cpool.tile([P, P], bf16, name="bsel_bf")
            nc.gpsimd.dma_start(
                out=bsel_bf.rearrange("(n r) m -> n r m", n=NB)[:, 0:2, :],
                in_=bsel[:],
            )

            # Per-image stationaries: h0 * tridiag and h1 * tridiag (bf16)
            tws = []
            for i in range(BPC):
                t0 = cpool.tile([P, P], bf16, name=f"tw0_{i}", tag=f"tw0_{i}")
                nc.vector.tensor_scalar_mul(
                    out=t0, in0=tri_bf, scalar1=hw_sb[:, 2 * i : 2 * i + 1]
                )
                t1 = cpool.tile([P, P], bf16, name=f"tw1_{i}", tag=f"tw1_{i}")
                nc.vector.tensor_scalar_mul(
                    out=t1, in0=tri_bf, scalar1=hw_sb[:, 2 * i + 1 : 2 * i + 2]
                )
                tws.append((t0, t1))

            for i in range(BPC):
                # First/last image: per-block loads (earlier PE start / earlier
                # tail compute); steady state: one 3 MB load per image for max
                # DMA efficiency.  Last image also stores per block.
                xi = xpool.tile([P, NB, FP], bf16, name="xi")
                nc.vector.memset(xi[:, :, 0:PAD], 0.0)
                nc.vector.memset(xi[:, :, F + PAD : FP], 0.0)
                if i == 0 or i == BPC - 1:
                    for n in range(NB):
                        nc.gpsimd.dma_start(
                            out=xi[:, n, PAD : F + PAD], in_=xr[i][:, n, :]
                        )
                else:
                    nc.gpsimd.dma_start(out=xi[:, :, PAD : F + PAD], in_=xr[i])

                # boundary pairs for block n at partitions 32n..32n+1
                pb = pbpool.tile([P, F], bf16, name="pb")
                nc.gpsimd.dma_start(
                    out=pb.rearrange("(n r) f -> n r f", n=NB)[:, 0:2, :],
                    in_=hbr[i],
                )

                oi = None
                if i < BPC - 1:
                    oi = opool.tile([P, NB, F], f32, name="oi", tag="oi")

                t0, t1 = tws[i]
                for n in range(NB):
                    pt = ppool.tile([P, F], f32, name="pt")
                    # taps at shifts 0 and 6 share the h0 stationary
                    for b in range(NBANK):
                        nc.tensor.matmul(
                            out=pt[:, b * BANK : (b + 1) * BANK],
                            lhsT=t0,
                            rhs=xi[:, n, b * BANK : b * BANK + BANK],
                            start=True,
                            stop=False,
                        )
                    for b in range(NBANK):
                        nc.tensor.matmul(
                            out=pt[:, b * BANK : (b + 1) * BANK],
                            lhsT=t0,
                            rhs=xi[:, n, b * BANK + 6 : b * BANK + 6 + BANK],
                            start=False,
                            stop=False,
                        )
                    for b in range(NBANK):
                        nc.tensor.matmul(
                            out=pt[:, b * BANK : (b + 1) * BANK],
                            lhsT=t1,
                            rhs=xi[:, n, b * BANK + 3 : b * BANK + 3 + BANK],
                            start=False,
                            stop=False,
                        )
                    for b in range(NBANK):
                        nc.tensor.matmul(
                            out=pt[:, b * BANK : (b + 1) * BANK],
                            lhsT=bsel_bf[32 * n : 32 * n + 2, :],
                            rhs=pb[32 * n : 32 * n + 2, b * BANK : (b + 1) * BANK],
                            start=False,
                            stop=True,
                            tile_position=(32 * n, 0),
                        )
                    # PSUM -> SBUF split across ScalarE (2 banks) + VectorE (1)
                    if i < BPC - 1:
                        nc.scalar.copy(out=oi[:, n, 0 : 2 * BANK], in_=pt[:, 0 : 2 * BANK])
                        nc.vector.tensor_copy(
                            out=oi[:, n, 2 * BANK : F], in_=pt[:, 2 * BANK : F]
                        )
                    else:
                        # last image: per-block stores to shorten the tail
                        ob = opool.tile([P, F], f32, name="ob", tag="ob", bufs=4)
                        nc.scalar.copy(out=ob[:, 0 : 2 * BANK], in_=pt[:, 0 : 2 * BANK])
                        nc.vector.tensor_copy(
                            out=ob[:, 2 * BANK : F], in_=pt[:, 2 * BANK : F]
                        )
                        nc.sync.dma_start(out=outr[i][:, n, :], in_=ob)
                if i < BPC - 1:
                    nc.sync.dma_start(out=outr[i], in_=oi)

    nc.compile()
    return nc


def _hfilt(rows, h0, h1):
    """Horizontal 3-tap filter of full-width rows. rows: (B', W, C) f32."""
    p = np.pad(rows, ((0, 0), (1, 1), (0, 0)))
    return (
        h0[:, None, None] * (p[:, :-2] + p[:, 2:]) + h1[:, None, None] * rows
    ).astype(np.float32)


def kernel(x, stds):
    global LAST_RESULTS
    x = np.ascontiguousarray(np.asarray(x), dtype=np.float32)
    stds = np.asarray(stds, dtype=np.float32)
    assert x.shape == (B, H, W, C) and stds.shape == (B,)

    # Per-image horizontal tap weights (f32, mirrors the reference math)
    s = (stds * np.float32(3.0)).astype(np.float32)
    with np.errstate(divide="ignore", over="ignore"):
        e = np.exp(-(np.float32(1.0) / (s * s))).astype(np.float32)
    den = (np.float32(3.0) * (np.float32(1.0) + np.float32(2.0) * e)).astype(np.float32)
    h0 = (e / den).astype(np.float32)
    h1 = (np.float32(1.0) / den).astype(np.float32)

    xf = x.reshape(B, H, F)

    # Host-prefiltered vertical halo rows: hbnd[:,n,0] = H(x[:, 128n-1]),
    # hbnd[:,n,1] = H(x[:, 128(n+1)]); zero where the halo row is outside.
    hbnd = np.zeros((B, NB, 2, F), np.float32)
    for n in range(1, NB):
        hbnd[:, n, 0] = _hfilt(x[:, P * n - 1], h0, h1).reshape(B, F)
    for n in range(0, NB - 1):
        hbnd[:, n, 1] = _hfilt(x[:, P * (n + 1)], h0, h1).reshape(B, F)

    tri_np = np.zeros((P, P), np.float32)
    idx = np.arange(P)
    tri_np[idx, idx] = 1.0
    tri_np[idx[:-1], idx[:-1] + 1] = 1.0
    tri_np[idx[:-1] + 1, idx[:-1]] = 1.0

    bsel_np = np.zeros((NB, 2, P), np.float32)
    bsel_np[:, 0, 0] = 1.0
    bsel_np[:, 1, P - 1] = 1.0

    in_maps = []
    for c in range(N_CORES):
        sl = slice(c * BPC, (c + 1) * BPC)
        hw_np = np.zeros((P, 2 * BPC), np.float32)
        hw_np[:, 0::2] = h0[sl][None, :]
        hw_np[:, 1::2] = h1[sl][None, :]
        in_maps.append(
            {
                "x": xf[sl],
                "hbnd": np.ascontiguousarray(hbnd[sl]),
                "hwts": hw_np,
                "tri": tri_np,
                "bsel": bsel_np,
            }
        )

    nc = _build_nc()
    trace = bool(int(os.environ.get("BLUR_TRACE", "0")))
    res = run_bass_kernel_spmd(
        nc, in_maps, core_ids=list(range(N_CORES)), trace=trace
    )
    LAST_RESULTS = res

    outs = [res.results[c]["out"].reshape(BPC, H, W, C) for c in range(N_CORES)]
    return np.concatenate(outs, axis=0).astype(np.float32)
